# revision 19
# baseline (speedup 1.0000x reference)
"""Trainium2 Bass kernel for soft K-means assignment (vq_codebook).

v3: f16 main product + f8 DoubleRow residual products.

x.c needs ~18 bits of precision for the T=0.1 softmax (the 2e-2 output
gate tolerates ~0.02 logit noise; fp32r's ~11-bit rounding gives 0.2).
Split on the host:  x = xh(f16) + xl,  c = ch(f16) + cl, and
    x.c ~= xh.ch  +  xl.ch  +  xh.cl      (xl.cl ~ 2^-22, dropped)
The two residual products carry ~2^-11-scale corrections, so f8e4m3
operands suffice (their own rounding lands at ~3e-3 logit noise), and
both pack the FULL d=256 contraction into ONE DoubleRow matmul each
(2 reduction elements per partition, 0.5 cycles/row):
    PE per tile = 2 f16 matmuls + 2 f8 DoubleRow matmuls ~ 640ns
vs 6 f16 matmuls (1278ns) before. Host pre-scales the f8 pairs by
2^+5/2^-5 (xl/ch) and 2^-6/2^+6 (xh/cl) so products are unscaled and
subnormal quantization stays harmless.

The ||c||^2/2 bias needs full precision. It is computed on device
(DVE unpack+add, ACT Square with 0.5 folded into the scale, Pool
partition_all_reduce) — a ~8us serial chain. To hide it, the first
SUB_TILES tiles apply the bias as an fp32 DVE subtract (needs only the
broadcast sum, ready ~2us earlier), and later tiles fold it into the
PSUM accumulation as ONE 2-partition f16 matmul whose rows are the
f16 hi/lo split of -csq/2 (f16 values pass the f16 matmul exactly, so
the bias lands with ~1e-5 error and no per-tile vector op).

Per tile steady state: PE 5 matmuls -> l/20 in PSUM; DVE max-reduce +
mx*-20; ACT exp(20*pl - 20*mx) with accumulated row sum; Pool
normalize_recip (out = e/sum; all cross-engine edges forward-only).
Scheduling: dummy-matmul chain at t~0 (PE p-state ramp resets on any
idle), 8-tile input groups fully prefetched, output DMAs on the scalar
queue emitted OUT_DELAY tiles late (a waiting DMA at the head of the
in-order ACT SEQ would block exp dispatch), last two output groups
tapered to 2 tiles to shorten the drain, and no mid-program tile-pool
scopes (closing a pool inserts an all-engine barrier).
"""

import numpy as np
import ml_dtypes
from contextlib import ExitStack

import concourse.bass as bass
import concourse.bacc as bacc
import concourse.mybir as mybir
import concourse.tile as tile
from concourse.bass_utils import run_bass_kernel_spmd

N_CORES = 8
B, S, D = 32, 1024, 256
K = 512
N_TOTAL = B * S                   # 32768
N_PER_CORE = N_TOTAL // N_CORES   # 4096
P = 128                           # partitions / rows per tile
N_TILES = N_PER_CORE // P         # 32
GIN = 8                           # tiles per input DMA group (1024 rows)
OUT_DELAY = 1                     # tiles between data-ready and out-DMA emit
SUB_TILES = 7                     # early tiles: bias via DVE subtract
N_WARM = 8                        # dummy matmuls bridging setup (p-state ramp)
TEMPERATURE = 0.1
# host-side f8 pre-scales (products must be unscaled: sxl*sch8=1, sxh8*scl=1)
SXL, SCH8 = 2.0 ** 5, 2.0 ** -5
SXH8, SCL = 2.0 ** -6, 2.0 ** 6

F32 = mybir.dt.float32
F16 = mybir.dt.float16
F8 = mybir.dt.float8e4

# output groups: 7x4 tiles then 2x2 (small final transfers shorten the tail)
OGROUPS = [1, 1, 2, 2] + [4] * 5 + [2, 2, 1, 1]
OSTART = np.cumsum([0] + OGROUPS).tolist()


def _ogroup(t):
    for gi, (s, n) in enumerate(zip(OSTART, OGROUPS)):
        if s <= t < s + n:
            return gi, t - s, n
    raise ValueError(t)


def build_program():
    nc = bacc.Bacc("TRN2", target_bir_lowering=False, debug=False)
    xh_in = nc.dram_tensor("xh", [D, N_PER_CORE], F16, kind="ExternalInput")
    xl8_in = nc.dram_tensor("xl8", [P, 2, N_PER_CORE], F8,
                            kind="ExternalInput")
    xh8_in = nc.dram_tensor("xh8", [P, 2, N_PER_CORE], F8,
                            kind="ExternalInput")
    ch_in = nc.dram_tensor("ch", [D, K], F16, kind="ExternalInput")
    cl8_in = nc.dram_tensor("cl8", [P, 2, K], F8, kind="ExternalInput")
    ch8_in = nc.dram_tensor("ch8", [P, 2, K], F8, kind="ExternalInput")
    out = nc.dram_tensor("out", [N_PER_CORE, K], F32, kind="ExternalOutput")

    nd = D // P  # 2 d-chunks

    with tile.TileContext(nc) as tc, ExitStack() as ctx:
        singles = ctx.enter_context(tc.tile_pool(name="singles", bufs=1))
        setup_ps = ctx.enter_context(
            tc.tile_pool(name="setup_ps", bufs=1, space="PSUM"))

        # PE warm-up chain (p-state ramp needs continuous PE activity)
        wrow = singles.tile([1, K], F16)
        nc.vector.memset(wrow[:], 0.0)
        warm_ps = setup_ps.tile([1, K], F32)
        for w in range(N_WARM):
            nc.tensor.matmul(warm_ps[:], wrow[:, 0:1], wrow[:],
                             start=True, stop=True)

        # ---- centroid tables ----
        cl8_sb = singles.tile([P, nd, K], F8)
        nc.sync.dma_start(out=cl8_sb[:], in_=cl8_in.ap())
        ch8_sb = singles.tile([P, nd, K], F8)
        nc.sync.dma_start(out=ch8_sb[:], in_=ch8_in.ap())
        ch_sb = singles.tile([P, nd, K], F16)
        nc.sync.dma_start(out=ch_sb[:],
                          in_=ch_in.ap().rearrange("(j p) k -> p j k", j=nd))

        # ---- bias chain: bias_bcast[p,k] = +csq_k/2 on every partition,
        # then f16 hi/lo rows of -csq/2 for the per-tile bias matmul ----
        import concourse.bass_isa as bass_isa
        cl32 = singles.tile([P, nd, K], F32)
        c32 = singles.tile([P, nd, K], F32)
        sq = singles.tile([P, nd, K], F32)
        csq2 = singles.tile([P, nd, K], F32)
        H = K // 2
        bias_bcast = singles.tile([P, K], F32)
        for j in range(nd):
            for h in range(2):
                ks = slice(h * H, (h + 1) * H)
                # cl = cl8 * 2^-6 (undo host pre-scale); c = ch + cl
                nc.vector.tensor_scalar_mul(cl32[:, j, ks],
                                            cl8_sb[:, j, ks], 1.0 / SCL)
                nc.vector.tensor_tensor(out=c32[:, j, ks],
                                        in0=ch_sb[:, j, ks],
                                        in1=cl32[:, j, ks],
                                        op=mybir.AluOpType.add)
                nc.scalar.activation(sq[:, j, ks], c32[:, j, ks],
                                     mybir.ActivationFunctionType.Square,
                                     scale=float(np.sqrt(0.5)))
                nc.gpsimd.partition_all_reduce(csq2[:, j, ks], sq[:, j, ks],
                                               channels=P,
                                               reduce_op=bass_isa.ReduceOp.add)
                if j == nd - 1:
                    nc.vector.tensor_tensor(out=bias_bcast[:, ks],
                                            in0=csq2[:, 0, ks],
                                            in1=csq2[:, 1, ks],
                                            op=mybir.AluOpType.add)
        # rows b1+b2 = f16 hi/lo split of -csq/2 (b1 f16-exact; residual 1e-5)
        negrow = singles.tile([1, K], F32)
        nc.vector.tensor_scalar_mul(negrow[:], bias_bcast[0:1, :], -1.0)
        b12 = singles.tile([1, 2 * K], F16)
        b1row = b12[:, 0:K]
        b2row = b12[:, K:2 * K]
        nc.vector.tensor_copy(b1row, negrow[:])
        nc.vector.tensor_tensor(out=b2row, in0=negrow[:], in1=b1row,
                                op=mybir.AluOpType.subtract)
        # engine ops cannot write SBUF partition offset 1, so the two rows
        # are staged through DRAM and loaded back as one [2, K] tile
        dram = ctx.enter_context(tc.tile_pool(name="dram", bufs=1,
                                              space="DRAM"))
        bdram = dram.tile([1, 2 * K], F16)
        nc.gpsimd.dma_start(out=bdram[:], in_=b12[:])
        bias2 = singles.tile([2, K], F16)
        nc.sync.dma_start(out=bias2[:],
                          in_=bdram[:].rearrange("o (two k) -> (o two) k",
                                                 two=2))
        ones2 = singles.tile([2, P], F16)
        nc.vector.memset(ones2[:], 1.0)

        # ---- main loop ----
        xhpool = ctx.enter_context(tc.tile_pool(name="xhpool", bufs=4))
        xl8pool = ctx.enter_context(tc.tile_pool(name="xl8pool", bufs=4))
        xh8pool = ctx.enter_context(tc.tile_pool(name="xh8pool", bufs=4))
        psum = ctx.enter_context(tc.tile_pool(name="psum", bufs=7,
                                              space="PSUM"))
        nlpool = ctx.enter_context(tc.tile_pool(name="nlpool", bufs=4))
        epool = ctx.enter_context(tc.tile_pool(name="epool", bufs=8))
        opool4 = ctx.enter_context(tc.tile_pool(name="opool4", bufs=3))
        opool2 = ctx.enter_context(tc.tile_pool(name="opool2", bufs=2))
        stats = ctx.enter_context(tc.tile_pool(name="stats", bufs=8))

        xh_sb = xl8_sb = xh8_sb = None
        o_sb = None
        o_tiles = {}   # out-group index -> (tile, size)
        pend = {}      # tile t -> (e_sb, s_sb)

        def norm(td):
            """Pool normalize_recip: out = e / sum (forward edges only)."""
            nonlocal o_sb
            e_sb, s_sb = pend.pop(td)
            gi, slot, size = _ogroup(td)
            if slot == 0:
                pool = opool4 if size == 4 else opool2
                o_sb = pool.tile([P, size, K], F32, tag=f"o{size}",
                                 name="o_sb")
                o_tiles[gi] = (o_sb, size)
            nc.gpsimd.normalize_recip(o_sb[:, slot, :], e_sb[:], s_sb[:])

        def flush(gi):
            ot, size = o_tiles.pop(gi)
            rows = slice(OSTART[gi] * P, (OSTART[gi] + size) * P)
            nc.scalar.dma_start(
                out=out.ap()[rows, :].rearrange("(jj p) k -> p jj k",
                                                jj=size),
                in_=ot[:])

        for t in range(N_TILES):
            g, tt = divmod(t, GIN)
            if tt == 0:
                cols = slice(g * GIN * P, (g + 1) * GIN * P)
                xh_sb = xhpool.tile([P, nd, GIN * P], F16, tag="xh",
                                    name="xh_sb")
                nc.sync.dma_start(
                    out=xh_sb[:],
                    in_=xh_in.ap()[:, cols].rearrange("(j p) n -> p j n",
                                                      j=nd))
                xl8_sb = xl8pool.tile([P, nd, GIN * P], F8, tag="xl8",
                                      name="xl8_sb")
                nc.sync.dma_start(out=xl8_sb[:], in_=xl8_in.ap()[:, :, cols])
                xh8_sb = xh8pool.tile([P, nd, GIN * P], F8, tag="xh8",
                                      name="xh8_sb")
                nc.sync.dma_start(out=xh8_sb[:], in_=xh8_in.ap()[:, :, cols])

            col = slice(tt * P, (tt + 1) * P)
            pl = psum.tile([P, K], F32, tag="pl", name="pl")
            for j in range(nd):
                nc.tensor.matmul(pl[:], xh_sb[:, j, col], ch_sb[:, j, :],
                                 start=(j == 0), stop=False)
            nc.tensor.matmul(pl[:], xl8_sb[:, :, col], ch8_sb[:],
                             start=False, stop=False,
                             perf_mode=mybir.MatmulPerfMode.DoubleRow)
            use_mm_bias = t >= SUB_TILES
            nc.tensor.matmul(pl[:], xh8_sb[:, :, col], cl8_sb[:],
                             start=False, stop=not use_mm_bias,
                             perf_mode=mybir.MatmulPerfMode.DoubleRow)
            if use_mm_bias:
                # l/20 = cross - csq/2 lands directly in PSUM (one
                # 2-partition f16 matmul adds both hi/lo bias rows exactly)
                nc.tensor.matmul(pl[:], ones2[:], bias2[:],
                                 start=False, stop=True)
                l20 = pl
            else:
                # bias via exact fp32 subtract (bias_bcast is ready ~2us
                # before the f16 bias rows)
                l20 = nlpool.tile([P, K], F32, tag="nl", name="nl")
                nc.vector.tensor_tensor(out=l20[:], in0=pl[:],
                                        in1=bias_bcast[:],
                                        op=mybir.AluOpType.subtract)

            mx = stats.tile([P, 1], F32, tag="mx", name="mx")
            nc.vector.tensor_reduce(out=mx[:], in_=l20[:],
                                    axis=mybir.AxisListType.X,
                                    op=mybir.AluOpType.max)
            mxn = stats.tile([P, 1], F32, tag="mxn", name="mxn")
            nc.vector.tensor_scalar_mul(mxn[:], mx[:], -2.0 / TEMPERATURE)

            e_sb = epool.tile([P, K], F32, tag="e", name="e")
            s_sb = stats.tile([P, 1], F32, tag="s", name="s")
            nc.scalar.activation(e_sb[:], l20[:],
                                 mybir.ActivationFunctionType.Exp,
                                 bias=mxn[:], scale=2.0 / TEMPERATURE,
                                 accum_out=s_sb[:])
            pend[t] = (e_sb, s_sb)

            if t > 0:
                norm(t - 1)
            tdone = t - 1 - OUT_DELAY   # tile whose norm ran OUT_DELAY ago
            if tdone >= 0:
                gi, slot, size = _ogroup(tdone)
                if slot == size - 1 and gi in o_tiles:
                    flush(gi)

        norm(N_TILES - 1)
        for gi in sorted(o_tiles):
            flush(gi)

    nc.compile()
    return nc


_CACHED_NC = None


def _prep_x(xT):
    """f16 hi + pre-scaled f8 residual/lo operands, DoubleRow-packed."""
    xh = xT.astype(np.float16)
    xl = xT - xh.astype(np.float32)
    xl8 = (xl * SXL).astype(ml_dtypes.float8_e4m3)
    xh8 = (xh.astype(np.float32) * SXH8).astype(ml_dtypes.float8_e4m3)

    def pack(a):   # [256, n] -> [128, 2, n], d = j*128 + p
        return np.ascontiguousarray(
            a.reshape(2, P, -1).transpose(1, 0, 2))

    return np.ascontiguousarray(xh), pack(xl8), pack(xh8)


def kernel(x, centroids):
    global _CACHED_NC
    if _CACHED_NC is None:
        _CACHED_NC = build_program()
    nc = _CACHED_NC

    xf = np.asarray(x, dtype=np.float32).reshape(N_TOTAL, D)
    cT = np.asarray(centroids, dtype=np.float32).T
    ch = cT.astype(np.float16)
    cl = cT - ch.astype(np.float32)
    cl8 = (cl * SCL).astype(ml_dtypes.float8_e4m3)
    ch8 = (ch.astype(np.float32) * SCH8).astype(ml_dtypes.float8_e4m3)

    def pack(a):
        return np.ascontiguousarray(a.reshape(2, P, -1).transpose(1, 0, 2))

    cmap = {"ch": np.ascontiguousarray(ch), "cl8": pack(cl8),
            "ch8": pack(ch8)}
    in_maps = []
    for i in range(N_CORES):
        xh, xl8, xh8 = _prep_x(xf[i * N_PER_CORE:(i + 1) * N_PER_CORE].T)
        in_maps.append({"xh": xh, "xl8": xl8, "xh8": xh8, **cmap})
    res = run_bass_kernel_spmd(nc, in_maps, core_ids=list(range(N_CORES)))
    outs = np.concatenate([r["out"] for r in res.results], axis=0)
    return outs.reshape(B, S, K)


# revision 20
# speedup vs baseline: 1.0112x; 1.0112x over previous
"""Trainium2 Bass kernel for soft K-means assignment (vq_codebook).

v3: f16 main product + f8 DoubleRow residual products.

x.c needs ~18 bits of precision for the T=0.1 softmax (the 2e-2 output
gate tolerates ~0.02 logit noise; fp32r's ~11-bit rounding gives 0.2).
Split on the host:  x = xh(f16) + xl,  c = ch(f16) + cl, and
    x.c ~= xh.ch  +  xl.ch  +  xh.cl      (xl.cl ~ 2^-22, dropped)
The two residual products carry ~2^-11-scale corrections, so f8e4m3
operands suffice (their own rounding lands at ~3e-3 logit noise), and
both pack the FULL d=256 contraction into ONE DoubleRow matmul each
(2 reduction elements per partition, 0.5 cycles/row):
    PE per tile = 2 f16 matmuls + 2 f8 DoubleRow matmuls ~ 640ns
vs 6 f16 matmuls (1278ns) before. Host pre-scales the f8 pairs by
2^+5/2^-5 (xl/ch) and 2^-6/2^+6 (xh/cl) so products are unscaled and
subnormal quantization stays harmless.

The ||c||^2/2 bias needs full precision. It is computed on device
(DVE unpack+add, ACT Square with 0.5 folded into the scale, Pool
partition_all_reduce) — a ~8us serial chain. To hide it, the first
SUB_TILES tiles apply the bias as an fp32 DVE subtract (needs only the
broadcast sum, ready ~2us earlier), and later tiles fold it into the
PSUM accumulation as ONE 2-partition f16 matmul whose rows are the
f16 hi/lo split of -csq/2 (f16 values pass the f16 matmul exactly, so
the bias lands with ~1e-5 error and no per-tile vector op).

Per tile steady state: PE 5 matmuls -> l/20 in PSUM; DVE max-reduce +
mx*-20; ACT exp(20*pl - 20*mx) with accumulated row sum; Pool
normalize_recip (out = e/sum; all cross-engine edges forward-only).
Scheduling: dummy-matmul chain at t~0 (PE p-state ramp resets on any
idle), 8-tile input groups fully prefetched, output DMAs on the scalar
queue emitted OUT_DELAY tiles late (a waiting DMA at the head of the
in-order ACT SEQ would block exp dispatch), last two output groups
tapered to 2 tiles to shorten the drain, and no mid-program tile-pool
scopes (closing a pool inserts an all-engine barrier).
"""

import numpy as np
import ml_dtypes
from contextlib import ExitStack

import concourse.bass as bass
import concourse.bacc as bacc
import concourse.mybir as mybir
import concourse.tile as tile
from concourse.bass_utils import run_bass_kernel_spmd

N_CORES = 8
B, S, D = 32, 1024, 256
K = 512
N_TOTAL = B * S                   # 32768
N_PER_CORE = N_TOTAL // N_CORES   # 4096
P = 128                           # partitions / rows per tile
N_TILES = N_PER_CORE // P         # 32
GIN = 8                           # tiles per input DMA group (1024 rows)
OUT_DELAY = 1                     # tiles between data-ready and out-DMA emit
SUB_TILES = 7                     # early tiles: bias via DVE subtract
N_WARM = 8                        # dummy matmuls bridging setup (p-state ramp)
TEMPERATURE = 0.1
# host-side f8 pre-scales (products must be unscaled: sxl*sch8=1, sxh8*scl=1)
SXL, SCH8 = 2.0 ** 5, 2.0 ** -5
SXH8, SCL = 2.0 ** -6, 2.0 ** 6

F32 = mybir.dt.float32
F16 = mybir.dt.float16
F8 = mybir.dt.float8e4

# output groups: 7x4 tiles then 2x2 (small final transfers shorten the tail)
OGROUPS = [1, 1, 2, 2] + [4] * 5 + [2, 2, 1, 1]
OSTART = np.cumsum([0] + OGROUPS).tolist()


def _ogroup(t):
    for gi, (s, n) in enumerate(zip(OSTART, OGROUPS)):
        if s <= t < s + n:
            return gi, t - s, n
    raise ValueError(t)


def build_program():
    nc = bacc.Bacc("TRN2", target_bir_lowering=False, debug=False)
    xh_in = nc.dram_tensor("xh", [D, N_PER_CORE], F16, kind="ExternalInput")
    xl8_in = nc.dram_tensor("xl8", [P, 2, N_PER_CORE], F8,
                            kind="ExternalInput")
    xh8_in = nc.dram_tensor("xh8", [P, 2, N_PER_CORE], F8,
                            kind="ExternalInput")
    ch_in = nc.dram_tensor("ch", [D, K], F16, kind="ExternalInput")
    cl8_in = nc.dram_tensor("cl8", [P, 2, K], F8, kind="ExternalInput")
    ch8_in = nc.dram_tensor("ch8", [P, 2, K], F8, kind="ExternalInput")
    out = nc.dram_tensor("out", [N_PER_CORE, K], F32, kind="ExternalOutput")

    nd = D // P  # 2 d-chunks

    with tile.TileContext(nc) as tc, ExitStack() as ctx:
        singles = ctx.enter_context(tc.tile_pool(name="singles", bufs=1))
        setup_ps = ctx.enter_context(
            tc.tile_pool(name="setup_ps", bufs=1, space="PSUM"))

        # PE warm-up chain (p-state ramp needs continuous PE activity)
        wrow = singles.tile([1, K], F16)
        nc.vector.memset(wrow[:], 0.0)
        warm_ps = setup_ps.tile([1, K], F32)
        for w in range(N_WARM):
            nc.tensor.matmul(warm_ps[:], wrow[:, 0:1], wrow[:],
                             start=True, stop=True)

        # ---- centroid tables ----
        cl8_sb = singles.tile([P, nd, K], F8)
        nc.sync.dma_start(out=cl8_sb[:], in_=cl8_in.ap())
        ch8_sb = singles.tile([P, nd, K], F8)
        nc.sync.dma_start(out=ch8_sb[:], in_=ch8_in.ap())
        ch_sb = singles.tile([P, nd, K], F16)
        nc.sync.dma_start(out=ch_sb[:],
                          in_=ch_in.ap().rearrange("(j p) k -> p j k", j=nd))

        # ---- bias chain: bias_bcast[p,k] = +csq_k/2 on every partition,
        # then f16 hi/lo rows of -csq/2 for the per-tile bias matmul ----
        import concourse.bass_isa as bass_isa
        cl32 = singles.tile([P, nd, K], F32)
        c32 = singles.tile([P, nd, K], F32)
        sq = singles.tile([P, nd, K], F32)
        csq2 = singles.tile([P, nd, K], F32)
        H = K // 2
        bias_bcast = singles.tile([P, K], F32)
        for j in range(nd):
            for h in range(2):
                ks = slice(h * H, (h + 1) * H)
                # cl = cl8 * 2^-6 (undo host pre-scale); c = ch + cl
                nc.vector.tensor_scalar_mul(cl32[:, j, ks],
                                            cl8_sb[:, j, ks], 1.0 / SCL)
                nc.vector.tensor_tensor(out=c32[:, j, ks],
                                        in0=ch_sb[:, j, ks],
                                        in1=cl32[:, j, ks],
                                        op=mybir.AluOpType.add)
                nc.scalar.activation(sq[:, j, ks], c32[:, j, ks],
                                     mybir.ActivationFunctionType.Square,
                                     scale=float(np.sqrt(0.5)))
                nc.gpsimd.partition_all_reduce(csq2[:, j, ks], sq[:, j, ks],
                                               channels=P,
                                               reduce_op=bass_isa.ReduceOp.add)
                if j == nd - 1:
                    nc.vector.tensor_tensor(out=bias_bcast[:, ks],
                                            in0=csq2[:, 0, ks],
                                            in1=csq2[:, 1, ks],
                                            op=mybir.AluOpType.add)
        # rows b1+b2 = f16 hi/lo split of -csq/2 (b1 f16-exact; residual 1e-5)
        negrow = singles.tile([1, K], F32)
        nc.vector.tensor_scalar_mul(negrow[:], bias_bcast[0:1, :], -1.0)
        b12 = singles.tile([1, 2 * K], F16)
        b1row = b12[:, 0:K]
        b2row = b12[:, K:2 * K]
        nc.vector.tensor_copy(b1row, negrow[:])
        nc.vector.tensor_tensor(out=b2row, in0=negrow[:], in1=b1row,
                                op=mybir.AluOpType.subtract)
        # engine ops cannot write SBUF partition offset 1, so the two rows
        # are staged through DRAM and loaded back as one [2, K] tile
        dram = ctx.enter_context(tc.tile_pool(name="dram", bufs=1,
                                              space="DRAM"))
        bdram = dram.tile([1, 2 * K], F16)
        nc.gpsimd.dma_start(out=bdram[:], in_=b12[:])
        bias2 = singles.tile([2, K], F16)
        nc.sync.dma_start(out=bias2[:],
                          in_=bdram[:].rearrange("o (two k) -> (o two) k",
                                                 two=2))
        ones2 = singles.tile([2, P], F16)
        nc.vector.memset(ones2[:], 1.0)

        # ---- main loop ----
        xhpool = ctx.enter_context(tc.tile_pool(name="xhpool", bufs=4))
        xl8pool = ctx.enter_context(tc.tile_pool(name="xl8pool", bufs=4))
        xh8pool = ctx.enter_context(tc.tile_pool(name="xh8pool", bufs=4))
        psum = ctx.enter_context(tc.tile_pool(name="psum", bufs=7,
                                              space="PSUM"))
        nlpool = ctx.enter_context(tc.tile_pool(name="nlpool", bufs=4))
        epool = ctx.enter_context(tc.tile_pool(name="epool", bufs=8))
        opool4 = ctx.enter_context(tc.tile_pool(name="opool4", bufs=3))
        opool2 = ctx.enter_context(tc.tile_pool(name="opool2", bufs=2))
        stats = ctx.enter_context(tc.tile_pool(name="stats", bufs=20))

        xh_sb = xl8_sb = xh8_sb = None
        o_sb = None
        o_tiles = {}   # out-group index -> (tile, size)
        pend = {}      # tile t -> (e_sb, s_sb)

        def norm(td):
            """Pool normalize_recip: out = e / sum (forward edges only)."""
            nonlocal o_sb
            e_sb, s_sb = pend.pop(td)
            gi, slot, size = _ogroup(td)
            if slot == 0:
                pool = opool4 if size == 4 else opool2
                o_sb = pool.tile([P, size, K], F32, tag=f"o{size}",
                                 name="o_sb")
                o_tiles[gi] = (o_sb, size)
            nc.gpsimd.normalize_recip(o_sb[:, slot, :], e_sb[:], s_sb[:])

        def flush(gi):
            ot, size = o_tiles.pop(gi)
            rows = slice(OSTART[gi] * P, (OSTART[gi] + size) * P)
            nc.scalar.dma_start(
                out=out.ap()[rows, :].rearrange("(jj p) k -> p jj k",
                                                jj=size),
                in_=ot[:])

        for t in range(N_TILES):
            g, tt = divmod(t, GIN)
            if tt == 0:
                cols = slice(g * GIN * P, (g + 1) * GIN * P)
                xh_sb = xhpool.tile([P, nd, GIN * P], F16, tag="xh",
                                    name="xh_sb")
                nc.sync.dma_start(
                    out=xh_sb[:],
                    in_=xh_in.ap()[:, cols].rearrange("(j p) n -> p j n",
                                                      j=nd))
                xl8_sb = xl8pool.tile([P, nd, GIN * P], F8, tag="xl8",
                                      name="xl8_sb")
                nc.sync.dma_start(out=xl8_sb[:], in_=xl8_in.ap()[:, :, cols])
                xh8_sb = xh8pool.tile([P, nd, GIN * P], F8, tag="xh8",
                                      name="xh8_sb")
                nc.sync.dma_start(out=xh8_sb[:], in_=xh8_in.ap()[:, :, cols])

            col = slice(tt * P, (tt + 1) * P)
            pl = psum.tile([P, K], F32, tag="pl", name="pl")
            for j in range(nd):
                nc.tensor.matmul(pl[:], xh_sb[:, j, col], ch_sb[:, j, :],
                                 start=(j == 0), stop=False)
            nc.tensor.matmul(pl[:], xl8_sb[:, :, col], ch8_sb[:],
                             start=False, stop=False,
                             perf_mode=mybir.MatmulPerfMode.DoubleRow)
            use_mm_bias = t >= SUB_TILES
            nc.tensor.matmul(pl[:], xh8_sb[:, :, col], cl8_sb[:],
                             start=False, stop=not use_mm_bias,
                             perf_mode=mybir.MatmulPerfMode.DoubleRow)
            if use_mm_bias:
                # l/20 = cross - csq/2 lands directly in PSUM (one
                # 2-partition f16 matmul adds both hi/lo bias rows exactly)
                nc.tensor.matmul(pl[:], ones2[:], bias2[:],
                                 start=False, stop=True)
                l20 = pl
            else:
                # bias via exact fp32 subtract (bias_bcast is ready ~2us
                # before the f16 bias rows)
                l20 = nlpool.tile([P, K], F32, tag="nl", name="nl")
                nc.vector.tensor_tensor(out=l20[:], in0=pl[:],
                                        in1=bias_bcast[:],
                                        op=mybir.AluOpType.subtract)

            mx = stats.tile([P, 1], F32, tag="mx", name="mx")
            nc.vector.tensor_reduce(out=mx[:], in_=l20[:],
                                    axis=mybir.AxisListType.X,
                                    op=mybir.AluOpType.max)
            mxn = stats.tile([P, 1], F32, tag="mxn", name="mxn")
            nc.vector.tensor_scalar_mul(mxn[:], mx[:], -2.0 / TEMPERATURE)

            e_sb = epool.tile([P, K], F32, tag="e", name="e")
            s_sb = stats.tile([P, 1], F32, tag="s", name="s")
            nc.scalar.activation(e_sb[:], l20[:],
                                 mybir.ActivationFunctionType.Exp,
                                 bias=mxn[:], scale=2.0 / TEMPERATURE,
                                 accum_out=s_sb[:])
            pend[t] = (e_sb, s_sb)

            if t > 0:
                norm(t - 1)
            tdone = t - 1 - OUT_DELAY   # tile whose norm ran OUT_DELAY ago
            if tdone >= 0:
                gi, slot, size = _ogroup(tdone)
                if slot == size - 1 and gi in o_tiles:
                    flush(gi)

        norm(N_TILES - 1)
        for gi in sorted(o_tiles):
            flush(gi)

    nc.compile()
    return nc


_CACHED_NC = None


def _prep_x(xT):
    """f16 hi + pre-scaled f8 residual/lo operands, DoubleRow-packed."""
    xh = xT.astype(np.float16)
    xl = xT - xh.astype(np.float32)
    xl8 = (xl * SXL).astype(ml_dtypes.float8_e4m3)
    xh8 = (xh.astype(np.float32) * SXH8).astype(ml_dtypes.float8_e4m3)

    def pack(a):   # [256, n] -> [128, 2, n], d = j*128 + p
        return np.ascontiguousarray(
            a.reshape(2, P, -1).transpose(1, 0, 2))

    return np.ascontiguousarray(xh), pack(xl8), pack(xh8)


def kernel(x, centroids):
    global _CACHED_NC
    if _CACHED_NC is None:
        _CACHED_NC = build_program()
    nc = _CACHED_NC

    xf = np.asarray(x, dtype=np.float32).reshape(N_TOTAL, D)
    cT = np.asarray(centroids, dtype=np.float32).T
    ch = cT.astype(np.float16)
    cl = cT - ch.astype(np.float32)
    cl8 = (cl * SCL).astype(ml_dtypes.float8_e4m3)
    ch8 = (ch.astype(np.float32) * SCH8).astype(ml_dtypes.float8_e4m3)

    def pack(a):
        return np.ascontiguousarray(a.reshape(2, P, -1).transpose(1, 0, 2))

    cmap = {"ch": np.ascontiguousarray(ch), "cl8": pack(cl8),
            "ch8": pack(ch8)}
    in_maps = []
    for i in range(N_CORES):
        xh, xl8, xh8 = _prep_x(xf[i * N_PER_CORE:(i + 1) * N_PER_CORE].T)
        in_maps.append({"xh": xh, "xl8": xl8, "xh8": xh8, **cmap})
    res = run_bass_kernel_spmd(nc, in_maps, core_ids=list(range(N_CORES)))
    outs = np.concatenate([r["out"] for r in res.results], axis=0)
    return outs.reshape(B, S, K)


# revision 21
# speedup vs baseline: 1.0142x; 1.0029x over previous
"""Trainium2 Bass kernel for soft K-means assignment (vq_codebook).

v3: f16 main product + f8 DoubleRow residual products.

x.c needs ~18 bits of precision for the T=0.1 softmax (the 2e-2 output
gate tolerates ~0.02 logit noise; fp32r's ~11-bit rounding gives 0.2).
Split on the host:  x = xh(f16) + xl,  c = ch(f16) + cl, and
    x.c ~= xh.ch  +  xl.ch  +  xh.cl      (xl.cl ~ 2^-22, dropped)
The two residual products carry ~2^-11-scale corrections, so f8e4m3
operands suffice (their own rounding lands at ~3e-3 logit noise), and
both pack the FULL d=256 contraction into ONE DoubleRow matmul each
(2 reduction elements per partition, 0.5 cycles/row):
    PE per tile = 2 f16 matmuls + 2 f8 DoubleRow matmuls ~ 640ns
vs 6 f16 matmuls (1278ns) before. Host pre-scales the f8 pairs by
2^+5/2^-5 (xl/ch) and 2^-6/2^+6 (xh/cl) so products are unscaled and
subnormal quantization stays harmless.

The ||c||^2/2 bias needs full precision. It is computed on device
(DVE unpack+add, ACT Square with 0.5 folded into the scale, Pool
partition_all_reduce) — a ~8us serial chain. To hide it, the first
SUB_TILES tiles apply the bias as an fp32 DVE subtract (needs only the
broadcast sum, ready ~2us earlier), and later tiles fold it into the
PSUM accumulation as ONE 2-partition f16 matmul whose rows are the
f16 hi/lo split of -csq/2 (f16 values pass the f16 matmul exactly, so
the bias lands with ~1e-5 error and no per-tile vector op).

Per tile steady state: PE 5 matmuls -> l/20 in PSUM; DVE max-reduce +
mx*-20; ACT exp(20*pl - 20*mx) with accumulated row sum; Pool
normalize_recip (out = e/sum; all cross-engine edges forward-only).
Scheduling: dummy-matmul chain at t~0 (PE p-state ramp resets on any
idle), 8-tile input groups fully prefetched, output DMAs on the scalar
queue emitted OUT_DELAY tiles late (a waiting DMA at the head of the
in-order ACT SEQ would block exp dispatch), last two output groups
tapered to 2 tiles to shorten the drain, and no mid-program tile-pool
scopes (closing a pool inserts an all-engine barrier).
"""

import numpy as np
import ml_dtypes
from contextlib import ExitStack

import concourse.bass as bass
import concourse.bacc as bacc
import concourse.mybir as mybir
import concourse.tile as tile
from concourse.bass_utils import run_bass_kernel_spmd

N_CORES = 8
B, S, D = 32, 1024, 256
K = 512
N_TOTAL = B * S                   # 32768
N_PER_CORE = N_TOTAL // N_CORES   # 4096
P = 128                           # partitions / rows per tile
N_TILES = N_PER_CORE // P         # 32
GIN = 8                           # tiles per input DMA group (1024 rows)
OUT_DELAY = 2                     # tiles between data-ready and out-DMA emit
SUB_TILES = 7                     # early tiles: bias via DVE subtract
N_WARM = 8                        # dummy matmuls bridging setup (p-state ramp)
TEMPERATURE = 0.1
# host-side f8 pre-scales (products must be unscaled: sxl*sch8=1, sxh8*scl=1)
SXL, SCH8 = 2.0 ** 5, 2.0 ** -5
SXH8, SCL = 2.0 ** -6, 2.0 ** 6

F32 = mybir.dt.float32
F16 = mybir.dt.float16
F8 = mybir.dt.float8e4

# output groups: 7x4 tiles then 2x2 (small final transfers shorten the tail)
OGROUPS = [1, 1, 2, 2] + [4] * 5 + [2, 2, 1, 1]
OSTART = np.cumsum([0] + OGROUPS).tolist()


def _ogroup(t):
    for gi, (s, n) in enumerate(zip(OSTART, OGROUPS)):
        if s <= t < s + n:
            return gi, t - s, n
    raise ValueError(t)


def build_program():
    nc = bacc.Bacc("TRN2", target_bir_lowering=False, debug=False)
    xh_in = nc.dram_tensor("xh", [D, N_PER_CORE], F16, kind="ExternalInput")
    xl8_in = nc.dram_tensor("xl8", [P, 2, N_PER_CORE], F8,
                            kind="ExternalInput")
    xh8_in = nc.dram_tensor("xh8", [P, 2, N_PER_CORE], F8,
                            kind="ExternalInput")
    ch_in = nc.dram_tensor("ch", [D, K], F16, kind="ExternalInput")
    cl8_in = nc.dram_tensor("cl8", [P, 2, K], F8, kind="ExternalInput")
    ch8_in = nc.dram_tensor("ch8", [P, 2, K], F8, kind="ExternalInput")
    out = nc.dram_tensor("out", [N_PER_CORE, K], F32, kind="ExternalOutput")

    nd = D // P  # 2 d-chunks

    with tile.TileContext(nc) as tc, ExitStack() as ctx:
        singles = ctx.enter_context(tc.tile_pool(name="singles", bufs=1))
        setup_ps = ctx.enter_context(
            tc.tile_pool(name="setup_ps", bufs=1, space="PSUM"))

        # PE warm-up chain (p-state ramp needs continuous PE activity)
        wrow = singles.tile([1, K], F16)
        nc.vector.memset(wrow[:], 0.0)
        warm_ps = setup_ps.tile([1, K], F32)
        for w in range(N_WARM):
            nc.tensor.matmul(warm_ps[:], wrow[:, 0:1], wrow[:],
                             start=True, stop=True)

        # ---- centroid tables ----
        cl8_sb = singles.tile([P, nd, K], F8)
        nc.sync.dma_start(out=cl8_sb[:], in_=cl8_in.ap())
        ch8_sb = singles.tile([P, nd, K], F8)
        nc.sync.dma_start(out=ch8_sb[:], in_=ch8_in.ap())
        ch_sb = singles.tile([P, nd, K], F16)
        nc.sync.dma_start(out=ch_sb[:],
                          in_=ch_in.ap().rearrange("(j p) k -> p j k", j=nd))

        # ---- bias chain: bias_bcast[p,k] = +csq_k/2 on every partition,
        # then f16 hi/lo rows of -csq/2 for the per-tile bias matmul ----
        import concourse.bass_isa as bass_isa
        cl32 = singles.tile([P, nd, K], F32)
        c32 = singles.tile([P, nd, K], F32)
        sq = singles.tile([P, nd, K], F32)
        csq2 = singles.tile([P, nd, K], F32)
        H = K // 2
        bias_bcast = singles.tile([P, K], F32)
        for j in range(nd):
            for h in range(2):
                ks = slice(h * H, (h + 1) * H)
                # cl = cl8 * 2^-6 (undo host pre-scale); c = ch + cl
                nc.vector.tensor_scalar_mul(cl32[:, j, ks],
                                            cl8_sb[:, j, ks], 1.0 / SCL)
                nc.vector.tensor_tensor(out=c32[:, j, ks],
                                        in0=ch_sb[:, j, ks],
                                        in1=cl32[:, j, ks],
                                        op=mybir.AluOpType.add)
                nc.scalar.activation(sq[:, j, ks], c32[:, j, ks],
                                     mybir.ActivationFunctionType.Square,
                                     scale=float(np.sqrt(0.5)))
                nc.gpsimd.partition_all_reduce(csq2[:, j, ks], sq[:, j, ks],
                                               channels=P,
                                               reduce_op=bass_isa.ReduceOp.add)
                if j == nd - 1:
                    nc.vector.tensor_tensor(out=bias_bcast[:, ks],
                                            in0=csq2[:, 0, ks],
                                            in1=csq2[:, 1, ks],
                                            op=mybir.AluOpType.add)
        # rows b1+b2 = f16 hi/lo split of -csq/2 (b1 f16-exact; residual 1e-5)
        negrow = singles.tile([1, K], F32)
        nc.vector.tensor_scalar_mul(negrow[:], bias_bcast[0:1, :], -1.0)
        b12 = singles.tile([1, 2 * K], F16)
        b1row = b12[:, 0:K]
        b2row = b12[:, K:2 * K]
        nc.vector.tensor_copy(b1row, negrow[:])
        nc.vector.tensor_tensor(out=b2row, in0=negrow[:], in1=b1row,
                                op=mybir.AluOpType.subtract)
        # engine ops cannot write SBUF partition offset 1, so the two rows
        # are staged through DRAM and loaded back as one [2, K] tile
        dram = ctx.enter_context(tc.tile_pool(name="dram", bufs=1,
                                              space="DRAM"))
        bdram = dram.tile([1, 2 * K], F16)
        nc.gpsimd.dma_start(out=bdram[:], in_=b12[:])
        bias2 = singles.tile([2, K], F16)
        nc.sync.dma_start(out=bias2[:],
                          in_=bdram[:].rearrange("o (two k) -> (o two) k",
                                                 two=2))
        ones2 = singles.tile([2, P], F16)
        nc.vector.memset(ones2[:], 1.0)

        # ---- main loop ----
        xhpool = ctx.enter_context(tc.tile_pool(name="xhpool", bufs=4))
        xl8pool = ctx.enter_context(tc.tile_pool(name="xl8pool", bufs=4))
        xh8pool = ctx.enter_context(tc.tile_pool(name="xh8pool", bufs=4))
        psum = ctx.enter_context(tc.tile_pool(name="psum", bufs=7,
                                              space="PSUM"))
        nlpool = ctx.enter_context(tc.tile_pool(name="nlpool", bufs=4))
        epool = ctx.enter_context(tc.tile_pool(name="epool", bufs=8))
        opool4 = ctx.enter_context(tc.tile_pool(name="opool4", bufs=3))
        opool2 = ctx.enter_context(tc.tile_pool(name="opool2", bufs=2))
        stats = ctx.enter_context(tc.tile_pool(name="stats", bufs=20))

        xh_sb = xl8_sb = xh8_sb = None
        o_sb = None
        o_tiles = {}   # out-group index -> (tile, size)
        pend = {}      # tile t -> (e_sb, s_sb)

        def norm(td):
            """Pool normalize_recip: out = e / sum (forward edges only)."""
            nonlocal o_sb
            e_sb, s_sb = pend.pop(td)
            gi, slot, size = _ogroup(td)
            if slot == 0:
                pool = opool4 if size == 4 else opool2
                o_sb = pool.tile([P, size, K], F32, tag=f"o{size}",
                                 name="o_sb")
                o_tiles[gi] = (o_sb, size)
            nc.gpsimd.normalize_recip(o_sb[:, slot, :], e_sb[:], s_sb[:])

        def flush(gi):
            ot, size = o_tiles.pop(gi)
            rows = slice(OSTART[gi] * P, (OSTART[gi] + size) * P)
            nc.scalar.dma_start(
                out=out.ap()[rows, :].rearrange("(jj p) k -> p jj k",
                                                jj=size),
                in_=ot[:])

        for t in range(N_TILES):
            g, tt = divmod(t, GIN)
            if tt == 0:
                cols = slice(g * GIN * P, (g + 1) * GIN * P)
                xh_sb = xhpool.tile([P, nd, GIN * P], F16, tag="xh",
                                    name="xh_sb")
                nc.sync.dma_start(
                    out=xh_sb[:],
                    in_=xh_in.ap()[:, cols].rearrange("(j p) n -> p j n",
                                                      j=nd))
                xl8_sb = xl8pool.tile([P, nd, GIN * P], F8, tag="xl8",
                                      name="xl8_sb")
                nc.sync.dma_start(out=xl8_sb[:], in_=xl8_in.ap()[:, :, cols])
                xh8_sb = xh8pool.tile([P, nd, GIN * P], F8, tag="xh8",
                                      name="xh8_sb")
                nc.sync.dma_start(out=xh8_sb[:], in_=xh8_in.ap()[:, :, cols])

            col = slice(tt * P, (tt + 1) * P)
            pl = psum.tile([P, K], F32, tag="pl", name="pl")
            for j in range(nd):
                nc.tensor.matmul(pl[:], xh_sb[:, j, col], ch_sb[:, j, :],
                                 start=(j == 0), stop=False)
            nc.tensor.matmul(pl[:], xl8_sb[:, :, col], ch8_sb[:],
                             start=False, stop=False,
                             perf_mode=mybir.MatmulPerfMode.DoubleRow)
            use_mm_bias = t >= SUB_TILES
            nc.tensor.matmul(pl[:], xh8_sb[:, :, col], cl8_sb[:],
                             start=False, stop=not use_mm_bias,
                             perf_mode=mybir.MatmulPerfMode.DoubleRow)
            if use_mm_bias:
                # l/20 = cross - csq/2 lands directly in PSUM (one
                # 2-partition f16 matmul adds both hi/lo bias rows exactly)
                nc.tensor.matmul(pl[:], ones2[:], bias2[:],
                                 start=False, stop=True)
                l20 = pl
            else:
                # bias via exact fp32 subtract (bias_bcast is ready ~2us
                # before the f16 bias rows)
                l20 = nlpool.tile([P, K], F32, tag="nl", name="nl")
                nc.vector.tensor_tensor(out=l20[:], in0=pl[:],
                                        in1=bias_bcast[:],
                                        op=mybir.AluOpType.subtract)

            mx = stats.tile([P, 1], F32, tag="mx", name="mx")
            nc.vector.tensor_reduce(out=mx[:], in_=l20[:],
                                    axis=mybir.AxisListType.X,
                                    op=mybir.AluOpType.max)
            mxn = stats.tile([P, 1], F32, tag="mxn", name="mxn")
            nc.vector.tensor_scalar_mul(mxn[:], mx[:], -2.0 / TEMPERATURE)

            e_sb = epool.tile([P, K], F32, tag="e", name="e")
            s_sb = stats.tile([P, 1], F32, tag="s", name="s")
            nc.scalar.activation(e_sb[:], l20[:],
                                 mybir.ActivationFunctionType.Exp,
                                 bias=mxn[:], scale=2.0 / TEMPERATURE,
                                 accum_out=s_sb[:])
            pend[t] = (e_sb, s_sb)

            if t > 0:
                norm(t - 1)
            tdone = t - 1 - OUT_DELAY   # tile whose norm ran OUT_DELAY ago
            if tdone >= 0:
                gi, slot, size = _ogroup(tdone)
                if slot == size - 1 and gi in o_tiles:
                    flush(gi)

        norm(N_TILES - 1)
        for gi in sorted(o_tiles):
            flush(gi)

    nc.compile()
    return nc


_CACHED_NC = None


def _prep_x(xT):
    """f16 hi + pre-scaled f8 residual/lo operands, DoubleRow-packed."""
    xh = xT.astype(np.float16)
    xl = xT - xh.astype(np.float32)
    xl8 = (xl * SXL).astype(ml_dtypes.float8_e4m3)
    xh8 = (xh.astype(np.float32) * SXH8).astype(ml_dtypes.float8_e4m3)

    def pack(a):   # [256, n] -> [128, 2, n], d = j*128 + p
        return np.ascontiguousarray(
            a.reshape(2, P, -1).transpose(1, 0, 2))

    return np.ascontiguousarray(xh), pack(xl8), pack(xh8)


def kernel(x, centroids):
    global _CACHED_NC
    if _CACHED_NC is None:
        _CACHED_NC = build_program()
    nc = _CACHED_NC

    xf = np.asarray(x, dtype=np.float32).reshape(N_TOTAL, D)
    cT = np.asarray(centroids, dtype=np.float32).T
    ch = cT.astype(np.float16)
    cl = cT - ch.astype(np.float32)
    cl8 = (cl * SCL).astype(ml_dtypes.float8_e4m3)
    ch8 = (ch.astype(np.float32) * SCH8).astype(ml_dtypes.float8_e4m3)

    def pack(a):
        return np.ascontiguousarray(a.reshape(2, P, -1).transpose(1, 0, 2))

    cmap = {"ch": np.ascontiguousarray(ch), "cl8": pack(cl8),
            "ch8": pack(ch8)}
    in_maps = []
    for i in range(N_CORES):
        xh, xl8, xh8 = _prep_x(xf[i * N_PER_CORE:(i + 1) * N_PER_CORE].T)
        in_maps.append({"xh": xh, "xl8": xl8, "xh8": xh8, **cmap})
    res = run_bass_kernel_spmd(nc, in_maps, core_ids=list(range(N_CORES)))
    outs = np.concatenate([r["out"] for r in res.results], axis=0)
    return outs.reshape(B, S, K)
